# revision 1
# baseline (speedup 1.0000x reference)
"""GQA attention (16 Q heads / 4 KV heads, RoPE, n=2048, d=64) on 8 trn2 cores.

Sharding: core c = (batch b=c//4, kv-group j=c%4). Each core owns 4 query
heads sharing one KV head, computes its partial output projection
(O_heads @ Wo_rows), and the host sums the 4 partials per batch.

All on-device layouts keep head_dim (or inner dim) on SBUF partitions so no
activation transposes are needed:
  qT [64, 4*2048]  (4 heads concatenated along free)
  kT [64, 2048]
  S^T [keys, queries] tiles from matmul(lhsT=kT_blk, rhs=qT_chunk)
  P^T = exp(S^T/8) on ACT
  O^T+denom from matmul(lhsT=V_aug[keys,65], rhs=P^T)  (ones col -> denom)
Matmul inputs are bf16 (1 cycle/row), accumulation fp32 in PSUM.
"""

import os
import sys
import functools

import numpy as np

sys.path.insert(0, "/opt/trn_rl_repo")

import concourse.bass as bass  # noqa: E402
import concourse.bacc as bacc  # noqa: E402
import concourse.tile as tile  # noqa: E402
import concourse.mybir as mybir  # noqa: E402
from concourse.masks import make_identity  # noqa: E402

F32 = mybir.dt.float32
BF16 = mybir.dt.bfloat16
EXP = mybir.ActivationFunctionType.Exp

B, N, DIM = 2, 2048, 1024
HEADS, KVH, D = 16, 4, 64
HPC = HEADS // KVH          # q heads per core = 4
SCALE = D ** -0.5           # 1/8
QTOT = HPC * N              # 8192 concatenated query columns
NKB = N // 128              # 16 key blocks
NDB = DIM // 128            # 8 contraction blocks for projections

LAST_RESULTS = {}           # test.py introspection


def build_kernel(nc, tc, io):
    from contextlib import ExitStack

    xt, wq, wkv, wo = io["xt"], io["wq"], io["wkv"], io["wo"]
    cost, sincat, out = io["cost"], io["sincat"], io["out"]

    es = ExitStack()
    consts = es.enter_context(tc.tile_pool(name="consts", bufs=1))
    ot_pool = es.enter_context(tc.tile_pool(name="ot", bufs=1))
    qk_pool = es.enter_context(tc.tile_pool(name="qk", bufs=1))

    # --- constants / weights in SBUF ---
    wq_sb = consts.tile([128, NDB, 2 * 128], BF16, tag="wq")      # 8KB/part
    wkv_sb = consts.tile([128, NDB, 128], BF16, tag="wkv")        # 4KB/part
    wo_sb = consts.tile([128, 2, DIM], BF16, tag="wo")            # 8KB/part
    cos_sb = consts.tile([64, N], F32, tag="cos")                # 8KB/part
    sin_sb = consts.tile([64, N], F32, tag="sin")                # 8KB/part
    id64 = consts.tile([64, 64], BF16, tag="id")
    nc.sync.dma_start(wq_sb, wq.transpose([1, 0, 2]))
    nc.sync.dma_start(wkv_sb, wkv.transpose([1, 0, 2]))
    nc.sync.dma_start(wo_sb, wo.transpose([1, 0, 2]))
    nc.sync.dma_start(cos_sb, cost)
    nc.sync.dma_start(sin_sb, sincat)
    make_identity(nc, id64)

    # --- activations ---
    qt_sb = qk_pool.tile([128, QTOT], BF16, tag="qt")             # 16KB/part
    kt_sb = qk_pool.tile([128, N], BF16, tag="kt")                # 4KB/part
    vaug_sb = qk_pool.tile([128, NKB, 128], BF16, tag="vaug")     # 4KB/part
    # zero the pad regions once: K rows 64:128 of qt/kt, V cols 65:128
    nc.gpsimd.memset(qt_sb[64:128, :], 0.0)
    nc.gpsimd.memset(kt_sb[64:128, :], 0.0)
    nc.gpsimd.memset(vaug_sb, 0.0)
    ot_sb = [
        ot_pool.tile([128, N], BF16, tag=f"ot{i}", name=f"ot{i}") for i in range(2)
    ]
    # vT staging aliases into ot_sb[0] (free until attention writes it)
    vt_sb = ot_sb[0][0:64, :]

    def rope(dst, src, ch, tmp_pool):
        """dst[64,512] (SBUF) <- RoPE(src[64,512] (PSUM)), position chunk ch."""
        cs = cos_sb[:, ch * 512:(ch + 1) * 512]
        sn = sincat_slice = sin_sb[:, ch * 512:(ch + 1) * 512]
        t1 = tmp_pool.tile([64, 512], F32, tag="t1")
        t2 = tmp_pool.tile([64, 512], F32, tag="t2")
        nc.vector.tensor_mul(t1, src, cs)
        nc.vector.tensor_mul(t2[0:32, :], src[32:64, :], sn[0:32, :])
        nc.vector.tensor_mul(t2[32:64, :], src[0:32, :], sn[32:64, :])
        nc.vector.tensor_add(dst, t1, t2)

    with (
        tc.tile_pool(name="xt", bufs=1) as xt_pool,
        tc.tile_pool(name="ropetmp", bufs=2) as rope_tmp,
        tc.tile_pool(name="pproj", bufs=3, space="PSUM") as pp,
    ):
        xt_sb = xt_pool.tile([128, NDB, N], BF16, tag="xt")       # 64KB/part
        for kb in range(NDB):
            for ch in range(4):
                nc.sync.dma_start(
                    xt_sb[:, kb, ch * 512:(ch + 1) * 512],
                    xt[kb, :, ch * 512:(ch + 1) * 512],
                )

        # KV projection (k rows 0:64, v rows 64:128 of the pack).
        for ch in range(4):
            pkv = pp.tile([128, 512], F32, tag="pj")
            for kb in range(NDB):
                nc.tensor.matmul(
                    pkv,
                    wkv_sb[:, kb, :],
                    xt_sb[:, kb, ch * 512:(ch + 1) * 512],
                    start=(kb == 0),
                    stop=(kb == NDB - 1),
                )
            rope(kt_sb[0:64, ch * 512:(ch + 1) * 512], pkv[0:64, :], ch, rope_tmp)
            nc.vector.tensor_copy(
                vt_sb[:, ch * 512:(ch + 1) * 512], pkv[64:128, :]
            )

        # V_aug: transpose vT -> [keys,64] blocks, append ones column.
        for t in range(NKB):
            ptr = pp.tile([128, 64], BF16, tag="pjt")
            nc.tensor.transpose(
                ptr[:, 0:64], vt_sb[:, t * 128:(t + 1) * 128], id64
            )
            nc.vector.tensor_copy(vaug_sb[:, t, 0:64], ptr[:, 0:64])
            nc.vector.memset(vaug_sb[:, t, 64:65], 1.0)

        # Q projection: 2 head-pairs, 4 n-chunks each.
        for pack in range(2):
            for ch in range(4):
                pq = pp.tile([128, 512], F32, tag="pj")
                for kb in range(NDB):
                    nc.tensor.matmul(
                        pq,
                        wq_sb[:, kb, pack * 128:(pack + 1) * 128],
                        xt_sb[:, kb, ch * 512:(ch + 1) * 512],
                        start=(kb == 0),
                        stop=(kb == NDB - 1),
                    )
                for hh in range(2):
                    h = pack * 2 + hh
                    rope(
                        qt_sb[0:64, h * N + ch * 512: h * N + (ch + 1) * 512],
                        pq[hh * 64:(hh + 1) * 64, :],
                        ch,
                        rope_tmp,
                    )

    # --- attention ---
    with (
        tc.tile_pool(name="small", bufs=2) as small,
        tc.tile_pool(name="ppool", bufs=3) as ppool,
        tc.tile_pool(name="psS", bufs=2, space="PSUM") as psS,
        tc.tile_pool(name="psO", bufs=2, space="PSUM") as psO,
    ):
        for qc in range(QTOT // 1024):  # 8 chunks of 1024 queries
            po_t = psO.tile([128, 1024], F32, tag="o")
            for kb in range(NKB):
                ps_t = psS.tile([128, 1024], F32, tag="s")
                for half in range(2):
                    nc.tensor.matmul(
                        ps_t[:, half * 512:(half + 1) * 512],
                        kt_sb[:, kb * 128:(kb + 1) * 128],
                        qt_sb[:, qc * 1024 + half * 512: qc * 1024 + (half + 1) * 512],
                        start=True,
                        stop=True,
                    )
                p_t = ppool.tile([128, 1024], BF16, tag="p")
                nc.scalar.activation(p_t, ps_t, EXP, bias=0.0, scale=SCALE)
                for half in range(2):
                    nc.tensor.matmul(
                        po_t[:, half * 512:(half + 1) * 512],
                        vaug_sb[:, kb, :],
                        p_t[:, half * 512:(half + 1) * 512],
                        start=(kb == 0),
                        stop=(kb == NKB - 1),
                        skip_group_check=True,
                    )
            # normalize: O^T / denom (denom on psum partition 64)
            h = qc // 2
            pair, row0 = h // 2, 64 * (h % 2)
            col0 = (qc % 2) * 1024
            rc = small.tile([1, 1024], F32, tag="rc")
            nc.vector.reciprocal(rc, po_t[64:65, :])
            bc = small.tile([64, 1024], F32, tag="bc")
            nc.gpsimd.partition_broadcast(bc, rc)
            nc.vector.tensor_mul(
                ot_sb[pair][row0:row0 + 64, col0:col0 + 1024],
                po_t[0:64, :],
                bc,
            )

    # --- output projection: out[q, :] = sum_pair O^T_pair.T @ Wo_pair ---
    with (
        tc.tile_pool(name="pout", bufs=3, space="PSUM") as pout,
        tc.tile_pool(name="ostage", bufs=3) as ostage,
    ):
        for qb in range(N // 128):
            for nch in range(2):
                pt = pout.tile([128, 512], F32, tag="po")
                for pair in range(2):
                    nc.tensor.matmul(
                        pt,
                        ot_sb[pair][:, qb * 128:(qb + 1) * 128],
                        wo_sb[:, pair, nch * 512:(nch + 1) * 512],
                        start=(pair == 0),
                        stop=(pair == 1),
                    )
                st = ostage.tile([128, 512], F32, tag="st")
                nc.vector.tensor_copy(st, pt)
                nc.sync.dma_start(
                    out[qb * 128:(qb + 1) * 128, nch * 512:(nch + 1) * 512], st
                )

    es.close()


def _rope_tables():
    inv_freq = 1.0 / (10000.0 ** (np.arange(0, D, 2, dtype=np.float64) / D))
    freqs = np.outer(np.arange(N, dtype=np.float64), inv_freq)  # [N, 32]
    cos_h = np.cos(freqs).astype(np.float32).T                  # [32, N]
    sin_h = np.sin(freqs).astype(np.float32).T                  # [32, N]
    cost = np.concatenate([cos_h, cos_h], 0)                    # [64, N]
    sincat = np.concatenate([-sin_h, sin_h], 0)                 # [64, N]
    return np.ascontiguousarray(cost), np.ascontiguousarray(sincat)


@functools.lru_cache(maxsize=1)
def _program():
    nc = bacc.Bacc(
        "TRN2", target_bir_lowering=False, debug=False, enable_asserts=False
    )
    io = {
        "xt": nc.dram_tensor("xt", [NDB, 128, N], BF16, kind="ExternalInput").ap(),
        "wq": nc.dram_tensor("wq", [NDB, 128, 256], BF16, kind="ExternalInput").ap(),
        "wkv": nc.dram_tensor("wkv", [NDB, 128, 128], BF16, kind="ExternalInput").ap(),
        "wo": nc.dram_tensor("wo", [2, 128, DIM], BF16, kind="ExternalInput").ap(),
        "cost": nc.dram_tensor("cost", [64, N], F32, kind="ExternalInput").ap(),
        "sincat": nc.dram_tensor("sincat", [64, N], F32, kind="ExternalInput").ap(),
        "out": nc.dram_tensor("out", [N, DIM], F32, kind="ExternalOutput").ap(),
    }
    with tile.TileContext(nc) as tc:
        build_kernel(nc, tc, io)
    nc.compile()
    return nc


def make_in_maps(x, Wq, Wkv, Wo):
    import ml_dtypes

    bf16 = ml_dtypes.bfloat16
    cost, sincat = _rope_tables()
    in_maps = []
    for c in range(8):
        b, j = c // 4, c % 4
        xt = np.ascontiguousarray(x[b].T).reshape(NDB, 128, N)
        wq_c = np.ascontiguousarray(Wq[:, 256 * j:256 * (j + 1)]).reshape(
            NDB, 128, 256
        )
        wkv_c = np.ascontiguousarray(
            np.concatenate(
                [Wkv[:, 64 * j:64 * (j + 1)],
                 Wkv[:, 256 + 64 * j:256 + 64 * (j + 1)]],
                axis=1,
            )
        ).reshape(NDB, 128, 128)
        wo_c = np.ascontiguousarray(Wo[256 * j:256 * (j + 1), :]).reshape(
            2, 128, DIM
        )
        in_maps.append(
            {
                "xt": xt.astype(bf16),
                "wq": wq_c.astype(bf16),
                "wkv": wkv_c.astype(bf16),
                "wo": wo_c.astype(bf16),
                "cost": cost,
                "sincat": sincat,
            }
        )
    return in_maps


def _install_ntff_hook():
    """Register the axon NTFF profiling hook that this image's antenv lacks."""
    import types

    if "antenv.axon_hooks" in sys.modules:
        return
    try:
        sys.path.append("/root/.axon_site")
        from trn_agent_boot.trn_boot import _ntff_profile_via_ctypes

        hook = _ntff_profile_via_ctypes("/opt/axon/libaxon_pjrt.so")
    except Exception:
        hook = None
    finally:
        try:
            sys.path.remove("/root/.axon_site")
        except ValueError:
            pass
    mod = types.ModuleType("antenv.axon_hooks")
    mod.get_axon_ntff_profile_hook = lambda: hook
    mod.set_axon_ntff_profile_hook = lambda h: None
    sys.modules["antenv.axon_hooks"] = mod
    # artifact upload needs bucket credentials this container lacks
    import concourse.bass_utils as bu

    bu.upload_artifacts = lambda tmpdir: "local://" + str(tmpdir)


def kernel(x, Wq, Wkv, Wo, bo):
    from concourse.bass_utils import run_bass_kernel_spmd

    _install_ntff_hook()
    nc = _program()
    in_maps = make_in_maps(x, Wq, Wkv, Wo)
    trace = bool(os.environ.get("KERNEL_TRACE"))
    res = run_bass_kernel_spmd(
        nc, in_maps, list(range(8)), trace=trace
    )
    LAST_RESULTS["res"] = res
    full = np.zeros((B, N, DIM), np.float32)
    for c in range(8):
        full[c // 4] += res.results[c]["out"]
    full += bo.astype(np.float32)
    return full

